# revision 4
# baseline (speedup 1.0000x reference)
"""KMultiHeadedAttention Trainium2 kernel.

Full-input contract: kernel(**inputs) takes the unsharded numpy inputs and
returns the full [4, 256, 2048] output. Core c = (batch b, n-half): each of
the 8 cores computes projections, attention and the output projection for
its 1024 query positions; output slices are disjoint (no collectives).

Engine plan per core (fp16 activations, f32 PSUM):
  PE   : q/k/v projections, scores ST = k^T q (m on partitions), PV with a
         persistent ones-column for the softmax denominator, ones-matmul
         broadcast of the reciprocal, output projection.
  Act  : exp — path A straight from 2-bank PSUM score tiles, path BA from
         SBUF products.
  Pool : gpsimd pow applies the multiplicative mask AFTER exp
         (e = exp(s)^mw with mw = mask*weight host-premultiplied) and on
         path BP computes exp itself as E^sm; gpsimd cannot read PSUM, so
         all of its inputs live in SBUF.
  DVE  : path-B sm = st*mw (PSUM read), PSUM evacuations, reciprocal and
         the divide-multiply.
Per block the 8 m-chunk pairs split A,BA,A,BP,A,BA,A,BA (one extra A pair
in the DMA-paced first two blocks), holding all four engines at 96-98%
occupancy in steady state. The v-bias folds into the output bias on the
host (softmax rows sum to 1), weights/inputs load as fused single DMAs
(HWDGE descriptor generation, ~650ns/instr serial, dominates startup), and
block 7 chases its own PV pairs (lag 3), halves its final pair's
elementwise width, and splits the last output projection by head with the
bias as a ones-row so the tail evacuations run on Act and DVE in parallel.
"""

import sys

sys.path.insert(0, "/opt/trn_rl_repo")

import numpy as np

B, D, N, M = 4, 256, 2048, 2048
H, HD = 4, 64
NCORES = 8
NH = N // 2
AUG = HD + 1  # 65: per-head vT columns incl. ones column
VA = H * AUG  # 260
VPAD = 66  # padded per-head va stride in vTa sbuf tile
F16 = np.float16

_PERM = np.array([4 * d + h for h in range(H) for d in range(HD)])  # c' -> old c

NA = 10  # m-chunks on path A (exp from PSUM + pool pow)
NB = 6  # m-chunks on path B (DVE mult + exp from SBUF)

_NC = None


def _build_nc():
    import concourse.bass as bass
    import concourse.tile as tile
    from concourse import mybir

    f32 = mybir.dt.float32
    f16 = mybir.dt.float16
    Alu = mybir.AluOpType
    Act = mybir.ActivationFunctionType

    nc = bass.Bass()
    # fused layouts: [128, ic*W + w] = orig[ic*128 + p, w]
    xq_d = nc.declare_dram_parameter("xq2", [128, 2 * NH], f16, isOutput=False)
    xk_d = nc.declare_dram_parameter("xk2", [128, 2 * M], f16, isOutput=False)
    xv_d = nc.declare_dram_parameter("xv2", [128, 2 * M], f16, isOutput=False)
    mw_d = nc.declare_dram_parameter("mw", [128, 2 * 16 * 512], f16, isOutput=False)
    wq_d = nc.declare_dram_parameter("wq2", [128, 2 * D], f16, isOutput=False)
    wk_d = nc.declare_dram_parameter("wk2", [128, 2 * D], f16, isOutput=False)
    wv_d = nc.declare_dram_parameter("wv2", [128, 2 * VA], f16, isOutput=False)
    wm_d = nc.declare_dram_parameter("wm2", [128, 2 * D], f16, isOutput=False)
    b6_d = nc.declare_dram_parameter("bias6", [128, 6], f32, isOutput=False)
    bmr_d = nc.declare_dram_parameter("bmrow", [1, D], f16, isOutput=False)
    out_d = nc.declare_dram_parameter("out", [D, NH], f32, isOutput=True)

    with tile.TileContext(nc) as tc:
        with (
            tc.tile_pool(name="consts", bufs=1) as consts,
            tc.tile_pool(name="pin", bufs=1) as pin,
            tc.tile_pool(name="persist", bufs=1) as persist,
            tc.tile_pool(name="work", bufs=2) as work,
            tc.tile_pool(name="ps", bufs=3, space="PSUM") as psum,
        ):
            # ------------- inputs & constants (k/q deps first) -------------
            # one fused contiguous DMA per tensor (HWDGE gen is ~650ns/instr,
            # serial: instruction count is the startup critical path)
            wk2 = consts.tile([128, 2 * D], f16, tag="wk2", name="wk2")
            xk2 = pin.tile([128, 2 * M], f16, tag="xk2", name="xk2")
            wq2 = consts.tile([128, 2 * D], f16, tag="wq2", name="wq2")
            xq2 = pin.tile([128, 2 * NH], f16, tag="xq2", name="xq2")
            wv2 = consts.tile([128, 2 * VA], f16, tag="wv2", name="wv2")
            xv2 = pin.tile([128, 2 * M], f16, tag="xv2", name="xv2")
            wm2 = consts.tile([128, 2 * D], f16, tag="wm2", name="wm2")
            b6_sb = consts.tile([128, 6], f32, tag="b6", name="b6")

            def half_ap(t, d, W, lo, hi):
                # [p, 2 ic, lo:hi] view of a fused [128, 2*W] tensor
                return t[:, :].rearrange("p (ic w) -> p ic w", ic=2)[:, :, lo:hi]

            mw_sb = []
            for g in range(2):
                t = work.tile([128, 16 * 512], f16, tag="mw", bufs=2, name=f"mw{g}")
                mw_sb.append(t)

            def load_mask_chunk(g, c):
                # mask rides the DVE DMA queue: its HWDGE + transfers overlap
                # the SP-queue input loads during the fill phase
                base = g * 16 * 512
                nc.sync.dma_start(
                    out=mw_sb[g][:, c * 2048 : (c + 1) * 2048],
                    in_=mw_d[:, base + c * 2048 : base + (c + 1) * 2048],
                )

            nc.sync.dma_start(out=wk2, in_=wk_d[:])
            nc.sync.dma_start(
                out=half_ap(xk2, xk_d, M, 0, 1024), in_=half_ap(xk_d, None, M, 0, 1024)
            )
            nc.sync.dma_start(out=b6_sb, in_=b6_d[:])
            nc.sync.dma_start(out=wq2, in_=wq_d[:])
            nc.sync.dma_start(
                out=half_ap(xq2, None, NH, 0, 512), in_=half_ap(xq_d, None, NH, 0, 512)
            )
            nc.sync.dma_start(out=wv2, in_=wv_d[:])
            nc.sync.dma_start(
                out=half_ap(xv2, None, M, 0, 512), in_=half_ap(xv_d, None, M, 0, 512)
            )
            nc.sync.dma_start(
                out=half_ap(xk2, None, M, 1024, 2048),
                in_=half_ap(xk_d, None, M, 1024, 2048),
            )
            load_mask_chunk(0, 0)
            nc.sync.dma_start(
                out=half_ap(xq2, None, NH, 512, 1024),
                in_=half_ap(xq_d, None, NH, 512, 1024),
            )
            nc.sync.dma_start(
                out=half_ap(xv2, None, M, 512, 1024),
                in_=half_ap(xv_d, None, M, 512, 1024),
            )
            load_mask_chunk(0, 1)
            nc.sync.dma_start(
                out=half_ap(xv2, None, M, 1024, 2048),
                in_=half_ap(xv_d, None, M, 1024, 2048),
            )
            nc.sync.dma_start(out=wm2, in_=wm_d[:])
            load_mask_chunk(0, 2)
            load_mask_chunk(0, 3)
            bmr_sb = consts.tile([1, D], f16, tag="bmr", name="bmr")
            nc.sync.dma_start(out=bmr_sb, in_=bmr_d[:])
            ones_sb = consts.tile([1, 512], f16, tag="ones", name="ones")
            nc.vector.memset(ones_sb, 1.0)

            bq_sb = b6_sb[:, 0:2]
            bk_sb = b6_sb[:, 2:4]
            bm_sb = b6_sb[:, 4:6]
            wk_sb = [wk2[:, i * D : (i + 1) * D] for i in range(2)]
            wq_sb = [wq2[:, i * D : (i + 1) * D] for i in range(2)]
            wv_sb = [wv2[:, i * VA : (i + 1) * VA] for i in range(2)]
            wm_sb = [wm2[:, i * D : (i + 1) * D] for i in range(2)]
            xk_sb = [xk2[:, i * M : (i + 1) * M] for i in range(2)]
            xq_sb = [xq2[:, i * NH : (i + 1) * NH] for i in range(2)]
            xv_sb = [xv2[:, i * M : (i + 1) * M] for i in range(2)]

            def load_mask(g):
                for c in range(4):
                    load_mask_chunk(g, c)

            # ---------------- persistent SBUF ----------------
            q_sb = [persist.tile([128, NH], f16, tag=f"q{i}", name=f"q{i}") for i in range(2)]
            k_sb = [persist.tile([128, M], f16, tag=f"k{i}", name=f"k{i}") for i in range(2)]
            x_sb = [persist.tile([128, NH], f16, tag=f"x{i}", name=f"x{i}") for i in range(2)]
            # vTa: [128 m, head, mc, va(padded)]
            vta = persist.tile([128, H * 16 * VPAD], f16, tag="vta", name="vta")
            vta_r = vta[:, :].rearrange("p (h m v) -> p h m v", h=H, m=16, v=VPAD)
            # denominator ones-column (col 64 of each head/mc), set once
            nc.vector.memset(vta_r[:, :, :, 64:65], 1.0)

            # Pre-touch consts on DVE so TS-encoded ops see their DMA sems
            # (after the dep-free memsets so they don't queue behind b6).
            pt = consts.tile([128, 8], f32, tag="pt", name="pt")
            nc.vector.tensor_copy(out=pt[:, 0:6], in_=b6_sb)

            def vta_ap(h, mc):
                # [128, 65] view for head h, m-chunk mc
                return vta_r[:, h, mc, 0:AUG]

            # ---------------- PE work generators ----------------
            def st_tile():
                return psum.tile([128, 1024], f32, tag="st", bufs=3, name="st")

            proj_jobs = []  # deferred PE jobs (callables) to sprinkle

            def q_proj(oc, g):
                ps = st_tile()
                for ic in range(2):
                    nc.tensor.matmul(
                        ps[:, 0:512],
                        lhsT=wq_sb[ic][:, oc * 128 : (oc + 1) * 128],
                        rhs=xq_sb[ic][:, g * 512 : (g + 1) * 512],
                        start=(ic == 0),
                        stop=(ic == 1),
                    )
                nc.vector.tensor_scalar_add(
                    out=q_sb[oc][:, g * 512 : (g + 1) * 512],
                    in0=ps[:, 0:512],
                    scalar1=bq_sb[:, oc : oc + 1],
                )

            def k_proj(oc, jp, split_evac=False):
                ps = st_tile()
                for jj in range(2):
                    j = jp * 2 + jj
                    for ic in range(2):
                        nc.tensor.matmul(
                            ps[:, jj * 512 : (jj + 1) * 512],
                            lhsT=wk_sb[ic][:, oc * 128 : (oc + 1) * 128],
                            rhs=xk_sb[ic][:, j * 512 : (j + 1) * 512],
                            start=(ic == 0),
                            stop=(ic == 1),
                        )
                    if split_evac:
                        nc.vector.tensor_scalar_add(
                            out=k_sb[oc][:, j * 512 : (j + 1) * 512],
                            in0=ps[:, jj * 512 : (jj + 1) * 512],
                            scalar1=bk_sb[:, oc : oc + 1],
                        )
                if not split_evac:
                    nc.vector.tensor_scalar_add(
                        out=k_sb[oc][:, jp * 1024 : (jp + 1) * 1024],
                        in0=ps,
                        scalar1=bk_sb[:, oc : oc + 1],
                    )

            def v_proj(mcp):  # mc pair (2 m-chunks of 128)
                ps = st_tile()
                for ii in range(2):
                    mc = mcp * 2 + ii
                    for ic in range(2):
                        nc.tensor.matmul(
                            ps[:, ii * 512 : ii * 512 + VA],
                            lhsT=xv_sb[ic][:, mc * 128 : (mc + 1) * 128],
                            rhs=wv_sb[ic],
                            start=(ic == 0),
                            stop=(ic == 1),
                        )
                # evac both chunks in one pass (v-bias folded into bm on
                # host: sum(prob)=1 makes it additive on x); alternate the
                # copy between DVE and Act to balance engine load
                in_ap = (
                    ps[:, :]
                    .rearrange("p (two x) -> p two x", two=2)[:, :, 0:VA]
                    .rearrange("p two (h v) -> p two h v", h=H, v=AUG)[:, :, :, 0:HD]
                )
                out_ap = vta_r[:, :, mcp * 2 : mcp * 2 + 2, 0:HD].transpose(
                    [0, 2, 1, 3]
                )
                nc.vector.tensor_copy(out=out_ap, in_=in_ap)

            # ---------------- attention ----------------
            # per-pair elementwise paths:
            #  A : Act exp(st) -> e0, Pool pow(e0, mw) -> e      (Act+Pool)
            #  BA: DVE st*mw -> sm,  Act exp(sm) -> e            (DVE+Act)
            #  BP: DVE st*mw -> sm,  Pool pow(E, sm) -> e        (DVE+Pool)
            PATHS_STEADY = ["A", "BA", "A", "BP", "A", "BA", "A", "BA"]
            # blocks 0-1 run the projection-evacuation jobs on DVE; shift one
            # B pair to A there to unload it
            PATHS_EARLY = ["A", "BA", "A", "BP", "A", "A", "A", "BA"]

            def paths_of(b):
                return PATHS_EARLY if b < 2 else PATHS_STEADY

            def idx_of(b):
                P = paths_of(b)
                ai = {p: i for i, p in enumerate([p for p in range(8) if P[p] == "A"])}
                bi = {p: i for i, p in enumerate([p for p in range(8) if P[p] != "A"])}
                return ai, bi

            NAP = 5  # max A pairs across block variants
            NBP = 4

            econst = consts.tile([128, 1024], f16, tag="econst", name="econst")
            nc.vector.memset(econst, 2.718281828459045)

            e_bufs = {}
            sm_bufs = {}
            xa_of = {}

            def alloc_block_bufs(b):
                e_bufs[b] = work.tile([128, 16 * 512], f16, tag="e", bufs=3, name="e")
                e_bufs[(b, "e0")] = work.tile(
                    [128, NAP * 1024], f16, tag="e0", bufs=3, name="e0"
                )
                sm_bufs[b] = work.tile([128, NBP * 1024], f16, tag="sm", bufs=3, name="sm")

            def emit_scores_pair(b, p):
                g, h = b // 4, b % 4
                oc, po = h // 2, 64 * (h % 2)
                g0 = g * 512
                st = st_tile()
                for ii in range(2):
                    mc = 2 * p + ii
                    nc.tensor.matmul(
                        st[:, ii * 512 : (ii + 1) * 512],
                        lhsT=k_sb[oc][po : po + 64, mc * 128 : (mc + 1) * 128],
                        rhs=q_sb[oc][po : po + 64, g0 : g0 + 512],
                        start=True,
                        stop=True,
                    )
                e_buf = e_bufs[b]
                A_IDX, B_IDX = idx_of(b)
                path = paths_of(b)[p]
                if path == "A":
                    ai = A_IDX[p]
                    e0 = e_bufs[(b, "e0")]
                    if b == 0 and p == 0:
                        # two 512-wide exps so Act starts on the first score mm
                        for hh in range(2):
                            nc.scalar.activation(
                                out=e0[:, hh * 512 : (hh + 1) * 512],
                                in_=st[:, hh * 512 : (hh + 1) * 512],
                                func=Act.Exp,
                            )
                    else:
                        nc.scalar.activation(
                            out=e0[:, ai * 1024 : (ai + 1) * 1024], in_=st, func=Act.Exp
                        )
                    nc.gpsimd.tensor_tensor(
                        out=e_buf[:, p * 1024 : (p + 1) * 1024],
                        in0=e0[:, ai * 1024 : (ai + 1) * 1024],
                        in1=mw_sb[g][:, p * 1024 : (p + 1) * 1024],
                        op=Alu.pow,
                    )
                else:
                    bi = B_IDX[p]
                    sm = sm_bufs[b]
                    if b == 7 and p == 7:
                        # split the kernel's last pair into 512-wide halves:
                        # the final m-chunk gates the whole drain chain
                        for hh in range(2):
                            nc.vector.tensor_tensor(
                                out=sm[:, bi * 1024 + hh * 512 : bi * 1024 + (hh + 1) * 512],
                                in0=st[:, hh * 512 : (hh + 1) * 512],
                                in1=mw_sb[g][:, p * 1024 + hh * 512 : p * 1024 + (hh + 1) * 512],
                                op=Alu.mult,
                            )
                            nc.scalar.activation(
                                out=e_buf[:, p * 1024 + hh * 512 : p * 1024 + (hh + 1) * 512],
                                in_=sm[:, bi * 1024 + hh * 512 : bi * 1024 + (hh + 1) * 512],
                                func=Act.Exp,
                            )
                        return
                    nc.vector.tensor_tensor(
                        out=sm[:, bi * 1024 : (bi + 1) * 1024],
                        in0=st,
                        in1=mw_sb[g][:, p * 1024 : (p + 1) * 1024],
                        op=Alu.mult,
                    )
                    if path == "BA":
                        nc.scalar.activation(
                            out=e_buf[:, p * 1024 : (p + 1) * 1024],
                            in_=sm[:, bi * 1024 : (bi + 1) * 1024],
                            func=Act.Exp,
                        )
                    else:
                        nc.gpsimd.tensor_tensor(
                            out=e_buf[:, p * 1024 : (p + 1) * 1024],
                            in0=econst,
                            in1=sm[:, bi * 1024 : (bi + 1) * 1024],
                            op=Alu.pow,
                        )

            def emit_pv_pair(b, p):
                g, h = b // 4, b % 4
                e_buf = e_bufs[b]
                xa = xa_of[b]
                for ii in range(2):
                    mc = 2 * p + ii
                    nc.tensor.matmul(
                        xa[0:AUG, :],
                        lhsT=vta_ap(h, mc),
                        rhs=e_buf[:, mc * 512 : (mc + 1) * 512],
                        start=(mc == 0),
                        stop=(mc == 15),
                    )

            def emit_recip(b):
                r = work.tile([1, 512], f16, tag="r", bufs=2, name="r")
                with nc.allow_low_precision(reason="fp16 recip: 5e-4 rel, fine"):
                    nc.vector.reciprocal(out=r, in_=xa_of[b][64:65, :])
                return r

            r_of = {}

            def emit_div_bcast(b):
                # broadcast 1/den into rows 64:128 of the same xa tile/bank
                nc.tensor.matmul(
                    xa_of[b][64:128, :],
                    lhsT=ones_sb[:, 0:64],
                    rhs=r_of[b],
                    start=True,
                    stop=True,
                )

            def emit_div_mult(b):
                g, h = b // 4, b % 4
                oc, po = h // 2, 64 * (h % 2)
                g0 = g * 512
                xa = xa_of[b]
                rb = work.tile([64, 512], f16, tag="rb", bufs=2, name="rb")
                nc.vector.tensor_copy(out=rb, in_=xa[64:128, :])
                nc.vector.tensor_tensor(
                    out=x_sb[oc][po : po + 64, g0 : g0 + 512],
                    in0=xa[0:64, :],
                    in1=rb,
                    op=Alu.mult,
                )

            def out_proj(oc, g):
                g0 = g * 512
                ps = st_tile()
                for cc in range(2):
                    nc.tensor.matmul(
                        ps[:, 0:512],
                        lhsT=wm_sb[cc][:, oc * 128 : (oc + 1) * 128],
                        rhs=x_sb[cc][:, g0 : g0 + 512],
                        start=(cc == 0),
                        stop=(cc == 1),
                    )
                ot = work.tile([128, 512], f32, tag="ot", bufs=2, name="ot")
                nc.vector.tensor_scalar_add(
                    out=ot, in0=ps[:, 0:512], scalar1=bm_sb[:, oc : oc + 1]
                )
                nc.sync.dma_start(
                    out=out_d[oc * 128 : (oc + 1) * 128, g0 : g0 + 512], in_=ot
                )

            # -------- schedule --------
            # PE warmup: dummy matmuls (never read) ramp the tensor engine
            # p-state to full clock while DMAs/evacuations land.
            def warm_burst(n):
                w = st_tile()
                for _ in range(n):
                    nc.tensor.matmul(
                        w[0:64, 0:64],
                        lhsT=ones_sb[:, 0:64],
                        rhs=ones_sb[:, 0:64],
                        start=True,
                        stop=True,
                        skip_group_check=True,
                    )

            warm_burst(60)
            # pre-phase: k oc0 (heads 0,1), q oc0 only; the rest sprinkled
            k_proj(0, 0, split_evac=True)
            q_proj(0, 0)

            proj_jobs = [
                lambda: k_proj(0, 1),
                lambda: v_proj(0),
                lambda: k_proj(1, 0),
                lambda: k_proj(1, 1),
                lambda: v_proj(1),
                lambda: q_proj(1, 0),
                lambda: v_proj(2),
                lambda: v_proj(3),
                lambda: q_proj(0, 1),
                lambda: v_proj(4),
                lambda: v_proj(5),
                lambda: q_proj(1, 1),
                lambda: v_proj(6),
                lambda: v_proj(7),
            ]

            def xa_tile():
                return psum.tile([128, 512], f32, tag="xa", bufs=2, name="xa")

            for b in range(8):
                if b == 2:
                    load_mask(1)
                alloc_block_bufs(b)
                for p in range(8):
                    emit_scores_pair(b, p)
                    if b > 0:
                        if p == 0:
                            xa_of[b - 1] = xa_tile()
                            if b >= 2:
                                emit_div_bcast(b - 2)
                        if p == 1:
                            if b >= 2:
                                emit_div_mult(b - 2)
                        emit_pv_pair(b - 1, p)
                        if p == 7:
                            r_of[b - 1] = emit_recip(b - 1)
                    if b == 7:
                        if p >= 3:
                            if p == 3:
                                xa_of[7] = xa_tile()
                            emit_pv_pair(7, p - 3)
                    if proj_jobs:
                        proj_jobs.pop(0)()
                # out-proj (oc, g0) needs xmult of blocks 0..3: emitted b5 p1
                if b == 5:
                    out_proj(0, 0)
                if b == 6:
                    out_proj(1, 0)
            # drain
            emit_div_bcast(6)
            emit_div_mult(6)
            for p in range(5, 8):
                emit_pv_pair(7, p)
            # partial out-proj for g1: everything except the h3 rows of x_sb[1]
            # (h3 = block 7 divides last); its 64-row matmul lands after.
            g0 = 512
            ps_oc = {}
            for oc in range(2):
                ps = st_tile()
                ps_oc[oc] = ps
                # bias folded in as a ones-row so the tail evacs are plain
                # copies that can run on Act and DVE in parallel
                nc.tensor.matmul(
                    ps[:, 0:512],
                    lhsT=bmr_sb[:, oc * 128 : (oc + 1) * 128],
                    rhs=ones_sb,
                    start=True,
                    stop=False,
                )
                nc.tensor.matmul(
                    ps[:, 0:512],
                    lhsT=wm_sb[0][:, oc * 128 : (oc + 1) * 128],
                    rhs=x_sb[0][:, g0 : g0 + 512],
                    start=False,
                    stop=False,
                )
                nc.tensor.matmul(
                    ps[:, 0:512],
                    lhsT=wm_sb[1][0:64, oc * 128 : (oc + 1) * 128],
                    rhs=x_sb[1][0:64, g0 : g0 + 512],
                    start=False,
                    stop=False,
                )
            r_of[7] = emit_recip(7)
            emit_div_bcast(7)
            emit_div_mult(7)
            ots = {}
            for oc in range(2):
                nc.tensor.matmul(
                    ps_oc[oc][:, 0:512],
                    lhsT=wm_sb[1][64:128, oc * 128 : (oc + 1) * 128],
                    rhs=x_sb[1][64:128, g0 : g0 + 512],
                    start=False,
                    stop=True,
                )
                ot = work.tile([128, 512], f32, tag="ot", bufs=2, name="ot")
                ots[oc] = ot
                if oc == 0:
                    nc.scalar.activation(out=ot, in_=ps_oc[oc][:, 0:512], func=Act.Copy)
                else:
                    nc.vector.tensor_copy(out=ot, in_=ps_oc[oc][:, 0:512])
            nc.sync.dma_start(out=out_d[0:128, g0 : g0 + 512], in_=ots[0])
            nc.sync.dma_start(out=out_d[128:256, g0 : g0 + 512], in_=ots[1])
    return nc


def _legalize_multi_waits(j):
    """Split >1 sync-waits per instruction into standalone EventSemaphore ops
    (this walrus build accepts at most one wait per TPB instruction)."""
    ctr = 0
    for f in j["functions"]:
        for b in f["blocks"]:
            out = []
            for inst in b["instructions"]:
                si = inst.get("sync_info")
                ow = (si or {}).get("on_wait") or []
                if len(ow) > 1:
                    for w in ow[:-1]:
                        ctr += 1
                        out.append(
                            {
                                "debug": inst.get("debug", 0),
                                "engine": inst["engine"],
                                "ins": [],
                                "name": f"legwait-{ctr}",
                                "opcode": "EventSemaphore",
                                "outs": [],
                                "sync_info": {"on_update": [], "on_wait": [w]},
                            }
                        )
                    si["on_wait"] = [ow[-1]]
                out.append(inst)
            b["instructions"] = out
    return j


def _get_nc():
    global _NC
    if _NC is None:
        import json as _json
        import types as _types

        nc = _build_nc()
        raw = nc.to_json_bytes()
        fixed = _json.dumps(_legalize_multi_waits(_json.loads(raw))).encode()
        nc.to_json_bytes = _types.MethodType(lambda self: fixed, nc)
        _NC = nc
    return _NC


def _prep_shards(inputs):
    f = lambda k: np.asarray(inputs[k], dtype=np.float32)
    q, k, v = f("query"), f("key"), f("value")
    w, mask = f("weight"), f("mask")
    Wq, bq = f("Wq"), f("bq")
    Wk, bk = f("Wk"), f("bk")
    Wv, bv = f("Wv"), f("bv")
    Wm, bm = f("Wm"), f("bm")

    def fuse(x):
        # [256, W] -> [128, 2*W] with [p, ic*W + w] = x[ic*128 + p, w]
        W = x.shape[1]
        return np.ascontiguousarray(
            x.reshape(2, 128, W).transpose(1, 0, 2).reshape(128, 2 * W)
        )

    p = _PERM
    wq2 = fuse((Wq[p] / 8.0).T.astype(F16))
    wk2 = fuse(Wk[p].T.astype(F16))
    WvTp = Wv[p].T
    wvT = np.zeros((D, VA), np.float32)
    for h in range(H):
        wvT[:, AUG * h : AUG * h + HD] = WvTp[:, HD * h : HD * (h + 1)]
    wv2 = fuse(wvT.astype(F16))
    wm2 = fuse(Wm[:, p].T.astype(F16))
    bqT = (bq[p] / 8.0).reshape(2, 128).T
    bkT = bk[p].reshape(2, 128).T
    # sum_m prob = 1, so the v bias is additive on x: fold into bm
    bmp = bm + Wm @ bv  # sum_m prob = 1 -> v bias is additive on x
    bmT = bmp.reshape(2, 128).T
    bias6 = np.ascontiguousarray(np.concatenate([bqT, bkT, bmT], axis=1))
    bmrow = np.ascontiguousarray(bmp.reshape(1, D)).astype(F16)

    qh, kh, vh = q.astype(F16), k.astype(F16), v.astype(F16)

    # per-batch tensors are shared by the two cores of each batch
    xk2_of = [fuse(kh[b]) for b in range(B)]
    xv2_of = [fuse(vh[b]) for b in range(B)]
    mwb_of = [(mask[b].T * w[b][:, None]).astype(F16) for b in range(B)]

    in_maps = []
    for c in range(NCORES):
        b, half = c // 2, c % 2
        n0 = half * NH
        sl = mwb_of[b][:, n0 : n0 + NH].reshape(16, 128, 2, 512)
        mw = np.ascontiguousarray(sl.transpose(1, 2, 0, 3)).reshape(128, 2 * 16 * 512)
        in_maps.append(
            dict(
                xq2=fuse(np.ascontiguousarray(qh[b, :, n0 : n0 + NH])),
                xk2=xk2_of[b],
                xv2=xv2_of[b],
                mw=mw,
                wq2=wq2,
                wk2=wk2,
                wv2=wv2,
                wm2=wm2,
                bias6=bias6,
                bmrow=bmrow,
            )
        )
    return in_maps


LAST_RESULT = None


def kernel(**inputs) -> np.ndarray:
    from concourse.bass_utils import run_bass_kernel_spmd

    in_maps = _prep_shards(inputs)
    nc = _get_nc()
    global LAST_RESULT
    LAST_RESULT = run_bass_kernel_spmd(nc, in_maps, core_ids=list(range(NCORES)))
    out = np.empty((B, D, N), np.float32)
    for c in range(NCORES):
        b, half = c // 2, c % 2
        out[b, :, half * NH : (half + 1) * NH] = LAST_RESULT.results[c]["out"]
    return out


# revision 5
# speedup vs baseline: 1.0050x; 1.0050x over previous
"""KMultiHeadedAttention Trainium2 kernel.

Full-input contract: kernel(**inputs) takes the unsharded numpy inputs and
returns the full [4, 256, 2048] output. Core c = (batch b, n-half): each of
the 8 cores computes projections, attention and the output projection for
its 1024 query positions; output slices are disjoint (no collectives).

Engine plan per core (fp16 activations, f32 PSUM):
  PE   : q/k/v projections, scores ST = k^T q (m on partitions), PV with a
         persistent ones-column for the softmax denominator, ones-matmul
         broadcast of the reciprocal, output projection.
  Act  : exp — path A straight from 2-bank PSUM score tiles, path BA from
         SBUF products.
  Pool : gpsimd pow applies the multiplicative mask AFTER exp
         (e = exp(s)^mw with mw = mask*weight host-premultiplied) and on
         path BP computes exp itself as E^sm; gpsimd cannot read PSUM, so
         all of its inputs live in SBUF.
  DVE  : path-B sm = st*mw (PSUM read), PSUM evacuations, reciprocal and
         the divide-multiply.
Per block the 8 m-chunk pairs split A,BA,A,BP,A,BA,A,BA (one extra A pair
in the DMA-paced first two blocks), holding all four engines at 96-98%
occupancy in steady state. The v-bias folds into the output bias on the
host (softmax rows sum to 1), weights/inputs load as fused single DMAs
(HWDGE descriptor generation, ~650ns/instr serial, dominates startup), and
block 7 chases its own PV pairs (lag 3), halves its final pair's
elementwise width, and splits the last output projection by head with the
bias as a ones-row so the tail evacuations run on Act and DVE in parallel.
The output ships as fp16 (host upcasts) to halve the drain-critical final
DMA.
"""

import sys

sys.path.insert(0, "/opt/trn_rl_repo")

import numpy as np

B, D, N, M = 4, 256, 2048, 2048
H, HD = 4, 64
NCORES = 8
NH = N // 2
AUG = HD + 1  # 65: per-head vT columns incl. ones column
VA = H * AUG  # 260
VPAD = 66  # padded per-head va stride in vTa sbuf tile
F16 = np.float16

_PERM = np.array([4 * d + h for h in range(H) for d in range(HD)])  # c' -> old c

NA = 10  # m-chunks on path A (exp from PSUM + pool pow)
NB = 6  # m-chunks on path B (DVE mult + exp from SBUF)

_NC = None


def _build_nc():
    import concourse.bass as bass
    import concourse.tile as tile
    from concourse import mybir

    f32 = mybir.dt.float32
    f16 = mybir.dt.float16
    Alu = mybir.AluOpType
    Act = mybir.ActivationFunctionType

    nc = bass.Bass()
    # fused layouts: [128, ic*W + w] = orig[ic*128 + p, w]
    xq_d = nc.declare_dram_parameter("xq2", [128, 2 * NH], f16, isOutput=False)
    xk_d = nc.declare_dram_parameter("xk2", [128, 2 * M], f16, isOutput=False)
    xv_d = nc.declare_dram_parameter("xv2", [128, 2 * M], f16, isOutput=False)
    mw_d = nc.declare_dram_parameter("mw", [128, 2 * 16 * 512], f16, isOutput=False)
    wq_d = nc.declare_dram_parameter("wq2", [128, 2 * D], f16, isOutput=False)
    wk_d = nc.declare_dram_parameter("wk2", [128, 2 * D], f16, isOutput=False)
    wv_d = nc.declare_dram_parameter("wv2", [128, 2 * VA], f16, isOutput=False)
    wm_d = nc.declare_dram_parameter("wm2", [128, 2 * D], f16, isOutput=False)
    b6_d = nc.declare_dram_parameter("bias6", [128, 6], f32, isOutput=False)
    bmr_d = nc.declare_dram_parameter("bmrow", [1, D], f16, isOutput=False)
    out_d = nc.declare_dram_parameter("out", [D, NH], f16, isOutput=True)

    with tile.TileContext(nc) as tc:
        with (
            tc.tile_pool(name="consts", bufs=1) as consts,
            tc.tile_pool(name="pin", bufs=1) as pin,
            tc.tile_pool(name="persist", bufs=1) as persist,
            tc.tile_pool(name="work", bufs=2) as work,
            tc.tile_pool(name="ps", bufs=3, space="PSUM") as psum,
        ):
            # ------------- inputs & constants (k/q deps first) -------------
            # one fused contiguous DMA per tensor (HWDGE gen is ~650ns/instr,
            # serial: instruction count is the startup critical path)
            wk2 = consts.tile([128, 2 * D], f16, tag="wk2", name="wk2")
            xk2 = pin.tile([128, 2 * M], f16, tag="xk2", name="xk2")
            wq2 = consts.tile([128, 2 * D], f16, tag="wq2", name="wq2")
            xq2 = pin.tile([128, 2 * NH], f16, tag="xq2", name="xq2")
            wv2 = consts.tile([128, 2 * VA], f16, tag="wv2", name="wv2")
            xv2 = pin.tile([128, 2 * M], f16, tag="xv2", name="xv2")
            wm2 = consts.tile([128, 2 * D], f16, tag="wm2", name="wm2")
            b6_sb = consts.tile([128, 6], f32, tag="b6", name="b6")

            def half_ap(t, d, W, lo, hi):
                # [p, 2 ic, lo:hi] view of a fused [128, 2*W] tensor
                return t[:, :].rearrange("p (ic w) -> p ic w", ic=2)[:, :, lo:hi]

            mw_sb = []
            for g in range(2):
                t = work.tile([128, 16 * 512], f16, tag="mw", bufs=2, name=f"mw{g}")
                mw_sb.append(t)

            def load_mask_chunk(g, c):
                # mask rides the DVE DMA queue: its HWDGE + transfers overlap
                # the SP-queue input loads during the fill phase
                base = g * 16 * 512
                nc.sync.dma_start(
                    out=mw_sb[g][:, c * 2048 : (c + 1) * 2048],
                    in_=mw_d[:, base + c * 2048 : base + (c + 1) * 2048],
                )

            nc.sync.dma_start(out=wk2, in_=wk_d[:])
            nc.sync.dma_start(
                out=half_ap(xk2, xk_d, M, 0, 1024), in_=half_ap(xk_d, None, M, 0, 1024)
            )
            nc.sync.dma_start(out=b6_sb, in_=b6_d[:])
            nc.sync.dma_start(out=wq2, in_=wq_d[:])
            nc.sync.dma_start(
                out=half_ap(xq2, None, NH, 0, 512), in_=half_ap(xq_d, None, NH, 0, 512)
            )
            nc.sync.dma_start(out=wv2, in_=wv_d[:])
            nc.sync.dma_start(
                out=half_ap(xv2, None, M, 0, 512), in_=half_ap(xv_d, None, M, 0, 512)
            )
            nc.sync.dma_start(
                out=half_ap(xk2, None, M, 1024, 2048),
                in_=half_ap(xk_d, None, M, 1024, 2048),
            )
            load_mask_chunk(0, 0)
            nc.sync.dma_start(
                out=half_ap(xq2, None, NH, 512, 1024),
                in_=half_ap(xq_d, None, NH, 512, 1024),
            )
            nc.sync.dma_start(
                out=half_ap(xv2, None, M, 512, 1024),
                in_=half_ap(xv_d, None, M, 512, 1024),
            )
            load_mask_chunk(0, 1)
            nc.sync.dma_start(
                out=half_ap(xv2, None, M, 1024, 2048),
                in_=half_ap(xv_d, None, M, 1024, 2048),
            )
            nc.sync.dma_start(out=wm2, in_=wm_d[:])
            load_mask_chunk(0, 2)
            load_mask_chunk(0, 3)
            bmr_sb = consts.tile([1, D], f16, tag="bmr", name="bmr")
            nc.sync.dma_start(out=bmr_sb, in_=bmr_d[:])
            ones_sb = consts.tile([1, 512], f16, tag="ones", name="ones")
            nc.vector.memset(ones_sb, 1.0)

            bq_sb = b6_sb[:, 0:2]
            bk_sb = b6_sb[:, 2:4]
            bm_sb = b6_sb[:, 4:6]
            wk_sb = [wk2[:, i * D : (i + 1) * D] for i in range(2)]
            wq_sb = [wq2[:, i * D : (i + 1) * D] for i in range(2)]
            wv_sb = [wv2[:, i * VA : (i + 1) * VA] for i in range(2)]
            wm_sb = [wm2[:, i * D : (i + 1) * D] for i in range(2)]
            xk_sb = [xk2[:, i * M : (i + 1) * M] for i in range(2)]
            xq_sb = [xq2[:, i * NH : (i + 1) * NH] for i in range(2)]
            xv_sb = [xv2[:, i * M : (i + 1) * M] for i in range(2)]

            def load_mask(g):
                for c in range(4):
                    load_mask_chunk(g, c)

            # ---------------- persistent SBUF ----------------
            q_sb = [persist.tile([128, NH], f16, tag=f"q{i}", name=f"q{i}") for i in range(2)]
            k_sb = [persist.tile([128, M], f16, tag=f"k{i}", name=f"k{i}") for i in range(2)]
            x_sb = [persist.tile([128, NH], f16, tag=f"x{i}", name=f"x{i}") for i in range(2)]
            # vTa: [128 m, head, mc, va(padded)]
            vta = persist.tile([128, H * 16 * VPAD], f16, tag="vta", name="vta")
            vta_r = vta[:, :].rearrange("p (h m v) -> p h m v", h=H, m=16, v=VPAD)
            # denominator ones-column (col 64 of each head/mc), set once
            nc.vector.memset(vta_r[:, :, :, 64:65], 1.0)

            # Pre-touch consts on DVE so TS-encoded ops see their DMA sems
            # (after the dep-free memsets so they don't queue behind b6).
            pt = consts.tile([128, 8], f32, tag="pt", name="pt")
            nc.vector.tensor_copy(out=pt[:, 0:6], in_=b6_sb)

            def vta_ap(h, mc):
                # [128, 65] view for head h, m-chunk mc
                return vta_r[:, h, mc, 0:AUG]

            # ---------------- PE work generators ----------------
            def st_tile():
                return psum.tile([128, 1024], f32, tag="st", bufs=3, name="st")

            proj_jobs = []  # deferred PE jobs (callables) to sprinkle

            def q_proj(oc, g):
                ps = st_tile()
                for ic in range(2):
                    nc.tensor.matmul(
                        ps[:, 0:512],
                        lhsT=wq_sb[ic][:, oc * 128 : (oc + 1) * 128],
                        rhs=xq_sb[ic][:, g * 512 : (g + 1) * 512],
                        start=(ic == 0),
                        stop=(ic == 1),
                    )
                nc.vector.tensor_scalar_add(
                    out=q_sb[oc][:, g * 512 : (g + 1) * 512],
                    in0=ps[:, 0:512],
                    scalar1=bq_sb[:, oc : oc + 1],
                )

            def k_proj(oc, jp, split_evac=False):
                ps = st_tile()
                for jj in range(2):
                    j = jp * 2 + jj
                    for ic in range(2):
                        nc.tensor.matmul(
                            ps[:, jj * 512 : (jj + 1) * 512],
                            lhsT=wk_sb[ic][:, oc * 128 : (oc + 1) * 128],
                            rhs=xk_sb[ic][:, j * 512 : (j + 1) * 512],
                            start=(ic == 0),
                            stop=(ic == 1),
                        )
                    if split_evac:
                        nc.vector.tensor_scalar_add(
                            out=k_sb[oc][:, j * 512 : (j + 1) * 512],
                            in0=ps[:, jj * 512 : (jj + 1) * 512],
                            scalar1=bk_sb[:, oc : oc + 1],
                        )
                if not split_evac:
                    nc.vector.tensor_scalar_add(
                        out=k_sb[oc][:, jp * 1024 : (jp + 1) * 1024],
                        in0=ps,
                        scalar1=bk_sb[:, oc : oc + 1],
                    )

            def v_proj(mcp):  # mc pair (2 m-chunks of 128)
                ps = st_tile()
                for ii in range(2):
                    mc = mcp * 2 + ii
                    for ic in range(2):
                        nc.tensor.matmul(
                            ps[:, ii * 512 : ii * 512 + VA],
                            lhsT=xv_sb[ic][:, mc * 128 : (mc + 1) * 128],
                            rhs=wv_sb[ic],
                            start=(ic == 0),
                            stop=(ic == 1),
                        )
                # evac both chunks in one pass (v-bias folded into bm on
                # host: sum(prob)=1 makes it additive on x); alternate the
                # copy between DVE and Act to balance engine load
                in_ap = (
                    ps[:, :]
                    .rearrange("p (two x) -> p two x", two=2)[:, :, 0:VA]
                    .rearrange("p two (h v) -> p two h v", h=H, v=AUG)[:, :, :, 0:HD]
                )
                out_ap = vta_r[:, :, mcp * 2 : mcp * 2 + 2, 0:HD].transpose(
                    [0, 2, 1, 3]
                )
                nc.vector.tensor_copy(out=out_ap, in_=in_ap)

            # ---------------- attention ----------------
            # per-pair elementwise paths:
            #  A : Act exp(st) -> e0, Pool pow(e0, mw) -> e      (Act+Pool)
            #  BA: DVE st*mw -> sm,  Act exp(sm) -> e            (DVE+Act)
            #  BP: DVE st*mw -> sm,  Pool pow(E, sm) -> e        (DVE+Pool)
            PATHS_STEADY = ["A", "BA", "A", "BP", "A", "BA", "A", "BA"]
            # blocks 0-1 run the projection-evacuation jobs on DVE; shift one
            # B pair to A there to unload it
            PATHS_EARLY = ["A", "BA", "A", "BP", "A", "A", "A", "BA"]

            def paths_of(b):
                return PATHS_EARLY if b < 2 else PATHS_STEADY

            def idx_of(b):
                P = paths_of(b)
                ai = {p: i for i, p in enumerate([p for p in range(8) if P[p] == "A"])}
                bi = {p: i for i, p in enumerate([p for p in range(8) if P[p] != "A"])}
                return ai, bi

            NAP = 5  # max A pairs across block variants
            NBP = 4

            econst = consts.tile([128, 1024], f16, tag="econst", name="econst")
            nc.vector.memset(econst, 2.718281828459045)

            e_bufs = {}
            sm_bufs = {}
            xa_of = {}

            def alloc_block_bufs(b):
                e_bufs[b] = work.tile([128, 16 * 512], f16, tag="e", bufs=3, name="e")
                e_bufs[(b, "e0")] = work.tile(
                    [128, NAP * 1024], f16, tag="e0", bufs=3, name="e0"
                )
                sm_bufs[b] = work.tile([128, NBP * 1024], f16, tag="sm", bufs=3, name="sm")

            def emit_scores_pair(b, p):
                g, h = b // 4, b % 4
                oc, po = h // 2, 64 * (h % 2)
                g0 = g * 512
                st = st_tile()
                for ii in range(2):
                    mc = 2 * p + ii
                    nc.tensor.matmul(
                        st[:, ii * 512 : (ii + 1) * 512],
                        lhsT=k_sb[oc][po : po + 64, mc * 128 : (mc + 1) * 128],
                        rhs=q_sb[oc][po : po + 64, g0 : g0 + 512],
                        start=True,
                        stop=True,
                    )
                e_buf = e_bufs[b]
                A_IDX, B_IDX = idx_of(b)
                path = paths_of(b)[p]
                if path == "A":
                    ai = A_IDX[p]
                    e0 = e_bufs[(b, "e0")]
                    if b == 0 and p == 0:
                        # two 512-wide exps so Act starts on the first score mm
                        for hh in range(2):
                            nc.scalar.activation(
                                out=e0[:, hh * 512 : (hh + 1) * 512],
                                in_=st[:, hh * 512 : (hh + 1) * 512],
                                func=Act.Exp,
                            )
                    else:
                        nc.scalar.activation(
                            out=e0[:, ai * 1024 : (ai + 1) * 1024], in_=st, func=Act.Exp
                        )
                    nc.gpsimd.tensor_tensor(
                        out=e_buf[:, p * 1024 : (p + 1) * 1024],
                        in0=e0[:, ai * 1024 : (ai + 1) * 1024],
                        in1=mw_sb[g][:, p * 1024 : (p + 1) * 1024],
                        op=Alu.pow,
                    )
                else:
                    bi = B_IDX[p]
                    sm = sm_bufs[b]
                    if b == 7 and p == 7:
                        # split the kernel's last pair into 512-wide halves:
                        # the final m-chunk gates the whole drain chain
                        for hh in range(2):
                            nc.vector.tensor_tensor(
                                out=sm[:, bi * 1024 + hh * 512 : bi * 1024 + (hh + 1) * 512],
                                in0=st[:, hh * 512 : (hh + 1) * 512],
                                in1=mw_sb[g][:, p * 1024 + hh * 512 : p * 1024 + (hh + 1) * 512],
                                op=Alu.mult,
                            )
                            nc.scalar.activation(
                                out=e_buf[:, p * 1024 + hh * 512 : p * 1024 + (hh + 1) * 512],
                                in_=sm[:, bi * 1024 + hh * 512 : bi * 1024 + (hh + 1) * 512],
                                func=Act.Exp,
                            )
                        return
                    nc.vector.tensor_tensor(
                        out=sm[:, bi * 1024 : (bi + 1) * 1024],
                        in0=st,
                        in1=mw_sb[g][:, p * 1024 : (p + 1) * 1024],
                        op=Alu.mult,
                    )
                    if path == "BA":
                        nc.scalar.activation(
                            out=e_buf[:, p * 1024 : (p + 1) * 1024],
                            in_=sm[:, bi * 1024 : (bi + 1) * 1024],
                            func=Act.Exp,
                        )
                    else:
                        nc.gpsimd.tensor_tensor(
                            out=e_buf[:, p * 1024 : (p + 1) * 1024],
                            in0=econst,
                            in1=sm[:, bi * 1024 : (bi + 1) * 1024],
                            op=Alu.pow,
                        )

            def emit_pv_pair(b, p):
                g, h = b // 4, b % 4
                e_buf = e_bufs[b]
                xa = xa_of[b]
                for ii in range(2):
                    mc = 2 * p + ii
                    nc.tensor.matmul(
                        xa[0:AUG, :],
                        lhsT=vta_ap(h, mc),
                        rhs=e_buf[:, mc * 512 : (mc + 1) * 512],
                        start=(mc == 0),
                        stop=(mc == 15),
                    )

            def emit_recip(b):
                r = work.tile([1, 512], f16, tag="r", bufs=2, name="r")
                with nc.allow_low_precision(reason="fp16 recip: 5e-4 rel, fine"):
                    nc.vector.reciprocal(out=r, in_=xa_of[b][64:65, :])
                return r

            r_of = {}

            def emit_div_bcast(b):
                # broadcast 1/den into rows 64:128 of the same xa tile/bank
                nc.tensor.matmul(
                    xa_of[b][64:128, :],
                    lhsT=ones_sb[:, 0:64],
                    rhs=r_of[b],
                    start=True,
                    stop=True,
                )

            def emit_div_mult(b):
                g, h = b // 4, b % 4
                oc, po = h // 2, 64 * (h % 2)
                g0 = g * 512
                xa = xa_of[b]
                rb = work.tile([64, 512], f16, tag="rb", bufs=2, name="rb")
                nc.vector.tensor_copy(out=rb, in_=xa[64:128, :])
                nc.vector.tensor_tensor(
                    out=x_sb[oc][po : po + 64, g0 : g0 + 512],
                    in0=xa[0:64, :],
                    in1=rb,
                    op=Alu.mult,
                )

            def out_proj(oc, g):
                g0 = g * 512
                ps = st_tile()
                for cc in range(2):
                    nc.tensor.matmul(
                        ps[:, 0:512],
                        lhsT=wm_sb[cc][:, oc * 128 : (oc + 1) * 128],
                        rhs=x_sb[cc][:, g0 : g0 + 512],
                        start=(cc == 0),
                        stop=(cc == 1),
                    )
                ot = work.tile([128, 512], f16, tag="ot", bufs=2, name="ot")
                nc.vector.tensor_scalar_add(
                    out=ot, in0=ps[:, 0:512], scalar1=bm_sb[:, oc : oc + 1]
                )
                nc.sync.dma_start(
                    out=out_d[oc * 128 : (oc + 1) * 128, g0 : g0 + 512], in_=ot
                )

            # -------- schedule --------
            # PE warmup: dummy matmuls (never read) ramp the tensor engine
            # p-state to full clock while DMAs/evacuations land.
            def warm_burst(n):
                w = st_tile()
                for _ in range(n):
                    nc.tensor.matmul(
                        w[0:64, 0:64],
                        lhsT=ones_sb[:, 0:64],
                        rhs=ones_sb[:, 0:64],
                        start=True,
                        stop=True,
                        skip_group_check=True,
                    )

            warm_burst(60)
            # pre-phase: k oc0 (heads 0,1), q oc0 only; the rest sprinkled
            k_proj(0, 0, split_evac=True)
            q_proj(0, 0)

            proj_jobs = [
                lambda: k_proj(0, 1),
                lambda: v_proj(0),
                lambda: k_proj(1, 0),
                lambda: k_proj(1, 1),
                lambda: v_proj(1),
                lambda: q_proj(1, 0),
                lambda: v_proj(2),
                lambda: v_proj(3),
                lambda: q_proj(0, 1),
                lambda: v_proj(4),
                lambda: v_proj(5),
                lambda: q_proj(1, 1),
                lambda: v_proj(6),
                lambda: v_proj(7),
            ]

            def xa_tile():
                return psum.tile([128, 512], f32, tag="xa", bufs=2, name="xa")

            for b in range(8):
                if b == 2:
                    load_mask(1)
                alloc_block_bufs(b)
                for p in range(8):
                    emit_scores_pair(b, p)
                    if b > 0:
                        if p == 0:
                            xa_of[b - 1] = xa_tile()
                            if b >= 2:
                                emit_div_bcast(b - 2)
                        if p == 1:
                            if b >= 2:
                                emit_div_mult(b - 2)
                        emit_pv_pair(b - 1, p)
                        if p == 7:
                            r_of[b - 1] = emit_recip(b - 1)
                    if b == 7:
                        if p >= 3:
                            if p == 3:
                                xa_of[7] = xa_tile()
                            emit_pv_pair(7, p - 3)
                    if proj_jobs:
                        proj_jobs.pop(0)()
                # out-proj (oc, g0) needs xmult of blocks 0..3: emitted b5 p1
                if b == 5:
                    out_proj(0, 0)
                if b == 6:
                    out_proj(1, 0)
            # drain
            emit_div_bcast(6)
            emit_div_mult(6)
            for p in range(5, 8):
                emit_pv_pair(7, p)
            # partial out-proj for g1: everything except the h3 rows of x_sb[1]
            # (h3 = block 7 divides last); its 64-row matmul lands after.
            g0 = 512
            ps_oc = {}
            for oc in range(2):
                ps = st_tile()
                ps_oc[oc] = ps
                # bias folded in as a ones-row so the tail evacs are plain
                # copies that can run on Act and DVE in parallel
                nc.tensor.matmul(
                    ps[:, 0:512],
                    lhsT=bmr_sb[:, oc * 128 : (oc + 1) * 128],
                    rhs=ones_sb,
                    start=True,
                    stop=False,
                )
                nc.tensor.matmul(
                    ps[:, 0:512],
                    lhsT=wm_sb[0][:, oc * 128 : (oc + 1) * 128],
                    rhs=x_sb[0][:, g0 : g0 + 512],
                    start=False,
                    stop=False,
                )
                nc.tensor.matmul(
                    ps[:, 0:512],
                    lhsT=wm_sb[1][0:64, oc * 128 : (oc + 1) * 128],
                    rhs=x_sb[1][0:64, g0 : g0 + 512],
                    start=False,
                    stop=False,
                )
            r_of[7] = emit_recip(7)
            emit_div_bcast(7)
            emit_div_mult(7)
            ots = {}
            for oc in range(2):
                nc.tensor.matmul(
                    ps_oc[oc][:, 0:512],
                    lhsT=wm_sb[1][64:128, oc * 128 : (oc + 1) * 128],
                    rhs=x_sb[1][64:128, g0 : g0 + 512],
                    start=False,
                    stop=True,
                )
                ot = work.tile([128, 512], f16, tag="ot", bufs=2, name="ot")
                ots[oc] = ot
                if oc == 0:
                    nc.scalar.activation(out=ot, in_=ps_oc[oc][:, 0:512], func=Act.Copy)
                else:
                    nc.vector.tensor_copy(out=ot, in_=ps_oc[oc][:, 0:512])
            nc.sync.dma_start(out=out_d[0:128, g0 : g0 + 512], in_=ots[0])
            nc.sync.dma_start(out=out_d[128:256, g0 : g0 + 512], in_=ots[1])
    return nc


def _legalize_multi_waits(j):
    """Split >1 sync-waits per instruction into standalone EventSemaphore ops
    (this walrus build accepts at most one wait per TPB instruction)."""
    ctr = 0
    for f in j["functions"]:
        for b in f["blocks"]:
            out = []
            for inst in b["instructions"]:
                si = inst.get("sync_info")
                ow = (si or {}).get("on_wait") or []
                if len(ow) > 1:
                    for w in ow[:-1]:
                        ctr += 1
                        out.append(
                            {
                                "debug": inst.get("debug", 0),
                                "engine": inst["engine"],
                                "ins": [],
                                "name": f"legwait-{ctr}",
                                "opcode": "EventSemaphore",
                                "outs": [],
                                "sync_info": {"on_update": [], "on_wait": [w]},
                            }
                        )
                    si["on_wait"] = [ow[-1]]
                out.append(inst)
            b["instructions"] = out
    return j


def _get_nc():
    global _NC
    if _NC is None:
        import json as _json
        import types as _types

        nc = _build_nc()
        raw = nc.to_json_bytes()
        fixed = _json.dumps(_legalize_multi_waits(_json.loads(raw))).encode()
        nc.to_json_bytes = _types.MethodType(lambda self: fixed, nc)
        _NC = nc
    return _NC


def _prep_shards(inputs):
    f = lambda k: np.asarray(inputs[k], dtype=np.float32)
    q, k, v = f("query"), f("key"), f("value")
    w, mask = f("weight"), f("mask")
    Wq, bq = f("Wq"), f("bq")
    Wk, bk = f("Wk"), f("bk")
    Wv, bv = f("Wv"), f("bv")
    Wm, bm = f("Wm"), f("bm")

    def fuse(x):
        # [256, W] -> [128, 2*W] with [p, ic*W + w] = x[ic*128 + p, w]
        W = x.shape[1]
        return np.ascontiguousarray(
            x.reshape(2, 128, W).transpose(1, 0, 2).reshape(128, 2 * W)
        )

    p = _PERM
    wq2 = fuse((Wq[p] / 8.0).T.astype(F16))
    wk2 = fuse(Wk[p].T.astype(F16))
    WvTp = Wv[p].T
    wvT = np.zeros((D, VA), np.float32)
    for h in range(H):
        wvT[:, AUG * h : AUG * h + HD] = WvTp[:, HD * h : HD * (h + 1)]
    wv2 = fuse(wvT.astype(F16))
    wm2 = fuse(Wm[:, p].T.astype(F16))
    bqT = (bq[p] / 8.0).reshape(2, 128).T
    bkT = bk[p].reshape(2, 128).T
    # sum_m prob = 1, so the v bias is additive on x: fold into bm
    bmp = bm + Wm @ bv  # sum_m prob = 1 -> v bias is additive on x
    bmT = bmp.reshape(2, 128).T
    bias6 = np.ascontiguousarray(np.concatenate([bqT, bkT, bmT], axis=1))
    bmrow = np.ascontiguousarray(bmp.reshape(1, D)).astype(F16)

    qh, kh, vh = q.astype(F16), k.astype(F16), v.astype(F16)

    # per-batch tensors are shared by the two cores of each batch
    xk2_of = [fuse(kh[b]) for b in range(B)]
    xv2_of = [fuse(vh[b]) for b in range(B)]
    mwb_of = [(mask[b].T * w[b][:, None]).astype(F16) for b in range(B)]

    in_maps = []
    for c in range(NCORES):
        b, half = c // 2, c % 2
        n0 = half * NH
        sl = mwb_of[b][:, n0 : n0 + NH].reshape(16, 128, 2, 512)
        mw = np.ascontiguousarray(sl.transpose(1, 2, 0, 3)).reshape(128, 2 * 16 * 512)
        in_maps.append(
            dict(
                xq2=fuse(np.ascontiguousarray(qh[b, :, n0 : n0 + NH])),
                xk2=xk2_of[b],
                xv2=xv2_of[b],
                mw=mw,
                wq2=wq2,
                wk2=wk2,
                wv2=wv2,
                wm2=wm2,
                bias6=bias6,
                bmrow=bmrow,
            )
        )
    return in_maps


LAST_RESULT = None


def kernel(**inputs) -> np.ndarray:
    from concourse.bass_utils import run_bass_kernel_spmd

    in_maps = _prep_shards(inputs)
    nc = _get_nc()
    global LAST_RESULT
    LAST_RESULT = run_bass_kernel_spmd(nc, in_maps, core_ids=list(range(NCORES)))
    out = np.empty((B, D, N), np.float32)
    for c in range(NCORES):
        b, half = c // 2, c % 2
        out[b, :, half * NH : (half + 1) * NH] = LAST_RESULT.results[c]["out"].astype(
            np.float32
        )
    return out


# revision 6
# speedup vs baseline: 1.0060x; 1.0010x over previous
"""KMultiHeadedAttention Trainium2 kernel.

Full-input contract: kernel(**inputs) takes the unsharded numpy inputs and
returns the full [4, 256, 2048] output. Core c = (batch b, n-half): each of
the 8 cores computes projections, attention and the output projection for
its 1024 query positions; output slices are disjoint (no collectives).

Engine plan per core (fp16 activations, f32 PSUM):
  PE   : q/k/v projections, scores ST = k^T q (m on partitions), PV with a
         persistent ones-column for the softmax denominator, ones-matmul
         broadcast of the reciprocal, output projection.
  Act  : exp — path A straight from 2-bank PSUM score tiles, path BA from
         SBUF products.
  Pool : gpsimd pow applies the multiplicative mask AFTER exp
         (e = exp(s)^mw with mw = mask*weight host-premultiplied) and on
         path BP computes exp itself as E^sm; gpsimd cannot read PSUM, so
         all of its inputs live in SBUF.
  DVE  : path-B sm = st*mw (PSUM read), PSUM evacuations, reciprocal and
         the divide-multiply.
Per block the 8 m-chunk pairs split A,BA,A,BP,A,BA,A,BA (one extra A pair
in the DMA-paced first two blocks), holding all four engines at 96-98%
occupancy in steady state. The v-bias folds into the output bias on the
host (softmax rows sum to 1), weights/inputs load as fused single DMAs
(HWDGE descriptor generation, ~650ns/instr serial, dominates startup), and
block 7 chases its own PV pairs (lag 3), halves its final pair's
elementwise width, and splits the last output projection by head with the
bias as a ones-row so the tail evacuations run on Act and DVE in parallel.
The output ships as fp16 (host upcasts) to halve the drain-critical final
DMA.
"""

import sys

sys.path.insert(0, "/opt/trn_rl_repo")

import numpy as np

B, D, N, M = 4, 256, 2048, 2048
H, HD = 4, 64
NCORES = 8
NH = N // 2
AUG = HD + 1  # 65: per-head vT columns incl. ones column
VA = H * AUG  # 260
VPAD = 66  # padded per-head va stride in vTa sbuf tile
F16 = np.float16

_PERM = np.array([4 * d + h for h in range(H) for d in range(HD)])  # c' -> old c

NA = 10  # m-chunks on path A (exp from PSUM + pool pow)
NB = 6  # m-chunks on path B (DVE mult + exp from SBUF)

_NC = None


def _build_nc():
    import concourse.bass as bass
    import concourse.tile as tile
    from concourse import mybir

    f32 = mybir.dt.float32
    f16 = mybir.dt.float16
    Alu = mybir.AluOpType
    Act = mybir.ActivationFunctionType

    nc = bass.Bass()
    # fused layouts: [128, ic*W + w] = orig[ic*128 + p, w]
    xq_d = nc.declare_dram_parameter("xq2", [128, 2 * NH], f16, isOutput=False)
    xk_d = nc.declare_dram_parameter("xk2", [128, 2 * M], f16, isOutput=False)
    xv_d = nc.declare_dram_parameter("xv2", [128, 2 * M], f16, isOutput=False)
    mw_d = nc.declare_dram_parameter("mw", [128, 2 * 16 * 512], f16, isOutput=False)
    wq_d = nc.declare_dram_parameter("wq2", [128, 2 * D], f16, isOutput=False)
    wk_d = nc.declare_dram_parameter("wk2", [128, 2 * D], f16, isOutput=False)
    wv_d = nc.declare_dram_parameter("wv2", [128, 2 * VA], f16, isOutput=False)
    wm_d = nc.declare_dram_parameter("wm2", [128, 2 * D], f16, isOutput=False)
    b6_d = nc.declare_dram_parameter("bias6", [128, 6], f32, isOutput=False)
    bmr_d = nc.declare_dram_parameter("bmrow", [1, D], f16, isOutput=False)
    out_d = nc.declare_dram_parameter("out", [D, NH], f16, isOutput=True)

    with tile.TileContext(nc) as tc:
        with (
            tc.tile_pool(name="consts", bufs=1) as consts,
            tc.tile_pool(name="pin", bufs=1) as pin,
            tc.tile_pool(name="persist", bufs=1) as persist,
            tc.tile_pool(name="work", bufs=2) as work,
            tc.tile_pool(name="ps", bufs=3, space="PSUM") as psum,
        ):
            # ------------- inputs & constants (k/q deps first) -------------
            # one fused contiguous DMA per tensor (HWDGE gen is ~650ns/instr,
            # serial: instruction count is the startup critical path)
            wk2 = consts.tile([128, 2 * D], f16, tag="wk2", name="wk2")
            xk2 = pin.tile([128, 2 * M], f16, tag="xk2", name="xk2")
            wq2 = consts.tile([128, 2 * D], f16, tag="wq2", name="wq2")
            xq2 = pin.tile([128, 2 * NH], f16, tag="xq2", name="xq2")
            wv2 = consts.tile([128, 2 * VA], f16, tag="wv2", name="wv2")
            xv2 = pin.tile([128, 2 * M], f16, tag="xv2", name="xv2")
            wm2 = consts.tile([128, 2 * D], f16, tag="wm2", name="wm2")
            b6_sb = consts.tile([128, 6], f32, tag="b6", name="b6")

            def half_ap(t, d, W, lo, hi):
                # [p, 2 ic, lo:hi] view of a fused [128, 2*W] tensor
                return t[:, :].rearrange("p (ic w) -> p ic w", ic=2)[:, :, lo:hi]

            mw_sb = []
            for g in range(2):
                t = work.tile([128, 16 * 512], f16, tag="mw", bufs=2, name=f"mw{g}")
                mw_sb.append(t)

            def load_mask_chunk(g, c):
                # mask rides the DVE DMA queue: its HWDGE + transfers overlap
                # the SP-queue input loads during the fill phase
                base = g * 16 * 512
                nc.sync.dma_start(
                    out=mw_sb[g][:, c * 2048 : (c + 1) * 2048],
                    in_=mw_d[:, base + c * 2048 : base + (c + 1) * 2048],
                )

            nc.sync.dma_start(out=wk2, in_=wk_d[:])
            nc.sync.dma_start(
                out=half_ap(xk2, xk_d, M, 0, 1024), in_=half_ap(xk_d, None, M, 0, 1024)
            )
            nc.sync.dma_start(out=b6_sb, in_=b6_d[:])
            nc.sync.dma_start(out=wq2, in_=wq_d[:])
            nc.sync.dma_start(
                out=half_ap(xq2, None, NH, 0, 512), in_=half_ap(xq_d, None, NH, 0, 512)
            )
            nc.sync.dma_start(out=wv2, in_=wv_d[:])
            nc.sync.dma_start(
                out=half_ap(xv2, None, M, 0, 512), in_=half_ap(xv_d, None, M, 0, 512)
            )
            nc.sync.dma_start(
                out=half_ap(xk2, None, M, 1024, 2048),
                in_=half_ap(xk_d, None, M, 1024, 2048),
            )
            load_mask_chunk(0, 0)
            nc.sync.dma_start(
                out=half_ap(xq2, None, NH, 512, 1024),
                in_=half_ap(xq_d, None, NH, 512, 1024),
            )
            nc.sync.dma_start(
                out=half_ap(xv2, None, M, 512, 1024),
                in_=half_ap(xv_d, None, M, 512, 1024),
            )
            load_mask_chunk(0, 1)
            nc.sync.dma_start(
                out=half_ap(xv2, None, M, 1024, 2048),
                in_=half_ap(xv_d, None, M, 1024, 2048),
            )
            nc.sync.dma_start(out=wm2, in_=wm_d[:])
            load_mask_chunk(0, 2)
            load_mask_chunk(0, 3)
            bmr_sb = consts.tile([1, D], f16, tag="bmr", name="bmr")
            nc.sync.dma_start(out=bmr_sb, in_=bmr_d[:])
            ones_sb = consts.tile([1, 512], f16, tag="ones", name="ones")
            nc.vector.memset(ones_sb, 1.0)

            bq_sb = b6_sb[:, 0:2]
            bk_sb = b6_sb[:, 2:4]
            bm_sb = b6_sb[:, 4:6]
            wk_sb = [wk2[:, i * D : (i + 1) * D] for i in range(2)]
            wq_sb = [wq2[:, i * D : (i + 1) * D] for i in range(2)]
            wv_sb = [wv2[:, i * VA : (i + 1) * VA] for i in range(2)]
            wm_sb = [wm2[:, i * D : (i + 1) * D] for i in range(2)]
            xk_sb = [xk2[:, i * M : (i + 1) * M] for i in range(2)]
            xq_sb = [xq2[:, i * NH : (i + 1) * NH] for i in range(2)]
            xv_sb = [xv2[:, i * M : (i + 1) * M] for i in range(2)]

            def load_mask(g):
                for c in range(4):
                    load_mask_chunk(g, c)

            # ---------------- persistent SBUF ----------------
            q_sb = [persist.tile([128, NH], f16, tag=f"q{i}", name=f"q{i}") for i in range(2)]
            k_sb = [persist.tile([128, M], f16, tag=f"k{i}", name=f"k{i}") for i in range(2)]
            x_sb = [persist.tile([128, NH], f16, tag=f"x{i}", name=f"x{i}") for i in range(2)]
            # vTa: [128 m, head, mc, va(padded)]
            vta = persist.tile([128, H * 16 * VPAD], f16, tag="vta", name="vta")
            vta_r = vta[:, :].rearrange("p (h m v) -> p h m v", h=H, m=16, v=VPAD)
            # denominator ones-column (col 64 of each head/mc), set once
            nc.vector.memset(vta_r[:, :, :, 64:65], 1.0)

            # Pre-touch consts on DVE so TS-encoded ops see their DMA sems
            # (after the dep-free memsets so they don't queue behind b6).
            pt = consts.tile([128, 8], f32, tag="pt", name="pt")
            nc.vector.tensor_copy(out=pt[:, 0:6], in_=b6_sb)

            def vta_ap(h, mc):
                # [128, 65] view for head h, m-chunk mc
                return vta_r[:, h, mc, 0:AUG]

            # ---------------- PE work generators ----------------
            def st_tile():
                return psum.tile([128, 1024], f32, tag="st", bufs=3, name="st")

            proj_jobs = []  # deferred PE jobs (callables) to sprinkle

            def q_proj(oc, g):
                ps = st_tile()
                for ic in range(2):
                    nc.tensor.matmul(
                        ps[:, 0:512],
                        lhsT=wq_sb[ic][:, oc * 128 : (oc + 1) * 128],
                        rhs=xq_sb[ic][:, g * 512 : (g + 1) * 512],
                        start=(ic == 0),
                        stop=(ic == 1),
                    )
                nc.vector.tensor_scalar_add(
                    out=q_sb[oc][:, g * 512 : (g + 1) * 512],
                    in0=ps[:, 0:512],
                    scalar1=bq_sb[:, oc : oc + 1],
                )

            def k_proj(oc, jp, split_evac=False):
                ps = st_tile()
                for jj in range(2):
                    j = jp * 2 + jj
                    for ic in range(2):
                        nc.tensor.matmul(
                            ps[:, jj * 512 : (jj + 1) * 512],
                            lhsT=wk_sb[ic][:, oc * 128 : (oc + 1) * 128],
                            rhs=xk_sb[ic][:, j * 512 : (j + 1) * 512],
                            start=(ic == 0),
                            stop=(ic == 1),
                        )
                    if split_evac:
                        nc.vector.tensor_scalar_add(
                            out=k_sb[oc][:, j * 512 : (j + 1) * 512],
                            in0=ps[:, jj * 512 : (jj + 1) * 512],
                            scalar1=bk_sb[:, oc : oc + 1],
                        )
                if not split_evac:
                    nc.vector.tensor_scalar_add(
                        out=k_sb[oc][:, jp * 1024 : (jp + 1) * 1024],
                        in0=ps,
                        scalar1=bk_sb[:, oc : oc + 1],
                    )

            def v_proj(mcp):  # mc pair (2 m-chunks of 128)
                ps = st_tile()
                for ii in range(2):
                    mc = mcp * 2 + ii
                    for ic in range(2):
                        nc.tensor.matmul(
                            ps[:, ii * 512 : ii * 512 + VA],
                            lhsT=xv_sb[ic][:, mc * 128 : (mc + 1) * 128],
                            rhs=wv_sb[ic],
                            start=(ic == 0),
                            stop=(ic == 1),
                        )
                # evac both chunks in one pass (v-bias folded into bm on
                # host: sum(prob)=1 makes it additive on x); alternate the
                # copy between DVE and Act to balance engine load
                in_ap = (
                    ps[:, :]
                    .rearrange("p (two x) -> p two x", two=2)[:, :, 0:VA]
                    .rearrange("p two (h v) -> p two h v", h=H, v=AUG)[:, :, :, 0:HD]
                )
                out_ap = vta_r[:, :, mcp * 2 : mcp * 2 + 2, 0:HD].transpose(
                    [0, 2, 1, 3]
                )
                nc.vector.tensor_copy(out=out_ap, in_=in_ap)

            # ---------------- attention ----------------
            # per-pair elementwise paths:
            #  A : Act exp(st) -> e0, Pool pow(e0, mw) -> e      (Act+Pool)
            #  BA: DVE st*mw -> sm,  Act exp(sm) -> e            (DVE+Act)
            #  BP: DVE st*mw -> sm,  Pool pow(E, sm) -> e        (DVE+Pool)
            PATHS_STEADY = ["A", "BA", "A", "BP", "A", "BA", "A", "BA"]
            # blocks 0-1 run the projection-evacuation jobs on DVE; shift one
            # B pair to A there to unload it
            PATHS_EARLY = ["A", "BA", "A", "BP", "A", "A", "A", "BA"]

            def paths_of(b):
                return PATHS_EARLY if b < 2 else PATHS_STEADY

            def idx_of(b):
                P = paths_of(b)
                ai = {p: i for i, p in enumerate([p for p in range(8) if P[p] == "A"])}
                bi = {p: i for i, p in enumerate([p for p in range(8) if P[p] != "A"])}
                return ai, bi

            NAP = 5  # max A pairs across block variants
            NBP = 4

            econst = consts.tile([128, 1024], f16, tag="econst", name="econst")
            nc.vector.memset(econst, 2.718281828459045)

            e_bufs = {}
            sm_bufs = {}
            xa_of = {}

            def alloc_block_bufs(b):
                e_bufs[b] = work.tile([128, 16 * 512], f16, tag="e", bufs=3, name="e")
                e_bufs[(b, "e0")] = work.tile(
                    [128, NAP * 1024], f16, tag="e0", bufs=3, name="e0"
                )
                sm_bufs[b] = work.tile([128, NBP * 1024], f16, tag="sm", bufs=3, name="sm")

            def emit_scores_pair(b, p):
                g, h = b // 4, b % 4
                oc, po = h // 2, 64 * (h % 2)
                g0 = g * 512
                st = st_tile()
                for ii in range(2):
                    mc = 2 * p + ii
                    nc.tensor.matmul(
                        st[:, ii * 512 : (ii + 1) * 512],
                        lhsT=k_sb[oc][po : po + 64, mc * 128 : (mc + 1) * 128],
                        rhs=q_sb[oc][po : po + 64, g0 : g0 + 512],
                        start=True,
                        stop=True,
                    )
                e_buf = e_bufs[b]
                A_IDX, B_IDX = idx_of(b)
                path = paths_of(b)[p]
                if path == "A":
                    ai = A_IDX[p]
                    e0 = e_bufs[(b, "e0")]
                    if b == 0 and p == 0:
                        # two 512-wide exps so Act starts on the first score mm
                        for hh in range(2):
                            nc.scalar.activation(
                                out=e0[:, hh * 512 : (hh + 1) * 512],
                                in_=st[:, hh * 512 : (hh + 1) * 512],
                                func=Act.Exp,
                            )
                    else:
                        nc.scalar.activation(
                            out=e0[:, ai * 1024 : (ai + 1) * 1024], in_=st, func=Act.Exp
                        )
                    nc.gpsimd.tensor_tensor(
                        out=e_buf[:, p * 1024 : (p + 1) * 1024],
                        in0=e0[:, ai * 1024 : (ai + 1) * 1024],
                        in1=mw_sb[g][:, p * 1024 : (p + 1) * 1024],
                        op=Alu.pow,
                    )
                else:
                    bi = B_IDX[p]
                    sm = sm_bufs[b]
                    if b == 7 and p == 7:
                        # split the kernel's last pair into 512-wide halves:
                        # the final m-chunk gates the whole drain chain
                        for hh in range(2):
                            nc.vector.tensor_tensor(
                                out=sm[:, bi * 1024 + hh * 512 : bi * 1024 + (hh + 1) * 512],
                                in0=st[:, hh * 512 : (hh + 1) * 512],
                                in1=mw_sb[g][:, p * 1024 + hh * 512 : p * 1024 + (hh + 1) * 512],
                                op=Alu.mult,
                            )
                            nc.scalar.activation(
                                out=e_buf[:, p * 1024 + hh * 512 : p * 1024 + (hh + 1) * 512],
                                in_=sm[:, bi * 1024 + hh * 512 : bi * 1024 + (hh + 1) * 512],
                                func=Act.Exp,
                            )
                        return
                    nc.vector.tensor_tensor(
                        out=sm[:, bi * 1024 : (bi + 1) * 1024],
                        in0=st,
                        in1=mw_sb[g][:, p * 1024 : (p + 1) * 1024],
                        op=Alu.mult,
                    )
                    if path == "BA":
                        nc.scalar.activation(
                            out=e_buf[:, p * 1024 : (p + 1) * 1024],
                            in_=sm[:, bi * 1024 : (bi + 1) * 1024],
                            func=Act.Exp,
                        )
                    else:
                        nc.gpsimd.tensor_tensor(
                            out=e_buf[:, p * 1024 : (p + 1) * 1024],
                            in0=econst,
                            in1=sm[:, bi * 1024 : (bi + 1) * 1024],
                            op=Alu.pow,
                        )

            def emit_pv_pair(b, p):
                g, h = b // 4, b % 4
                e_buf = e_bufs[b]
                xa = xa_of[b]
                for ii in range(2):
                    mc = 2 * p + ii
                    nc.tensor.matmul(
                        xa[0:AUG, :],
                        lhsT=vta_ap(h, mc),
                        rhs=e_buf[:, mc * 512 : (mc + 1) * 512],
                        start=(mc == 0),
                        stop=(mc == 15),
                    )

            def emit_recip(b):
                r = work.tile([1, 512], f16, tag="r", bufs=2, name="r")
                with nc.allow_low_precision(reason="fp16 recip: 5e-4 rel, fine"):
                    nc.vector.reciprocal(out=r, in_=xa_of[b][64:65, :])
                return r

            r_of = {}

            def emit_div_bcast(b):
                # broadcast 1/den into rows 64:128 of the same xa tile/bank
                nc.tensor.matmul(
                    xa_of[b][64:128, :],
                    lhsT=ones_sb[:, 0:64],
                    rhs=r_of[b],
                    start=True,
                    stop=True,
                )

            def emit_div_mult(b):
                g, h = b // 4, b % 4
                oc, po = h // 2, 64 * (h % 2)
                g0 = g * 512
                xa = xa_of[b]
                rb = work.tile([64, 512], f16, tag="rb", bufs=2, name="rb")
                nc.vector.tensor_copy(out=rb, in_=xa[64:128, :])
                nc.vector.tensor_tensor(
                    out=x_sb[oc][po : po + 64, g0 : g0 + 512],
                    in0=xa[0:64, :],
                    in1=rb,
                    op=Alu.mult,
                )

            def out_proj(oc, g):
                g0 = g * 512
                ps = st_tile()
                for cc in range(2):
                    nc.tensor.matmul(
                        ps[:, 0:512],
                        lhsT=wm_sb[cc][:, oc * 128 : (oc + 1) * 128],
                        rhs=x_sb[cc][:, g0 : g0 + 512],
                        start=(cc == 0),
                        stop=(cc == 1),
                    )
                ot = work.tile([128, 512], f16, tag="ot", bufs=2, name="ot")
                nc.vector.tensor_scalar_add(
                    out=ot, in0=ps[:, 0:512], scalar1=bm_sb[:, oc : oc + 1]
                )
                nc.sync.dma_start(
                    out=out_d[oc * 128 : (oc + 1) * 128, g0 : g0 + 512], in_=ot
                )

            # -------- schedule --------
            # PE warmup: dummy matmuls (never read) ramp the tensor engine
            # p-state to full clock while DMAs/evacuations land.
            def warm_burst(n):
                w = st_tile()
                for _ in range(n):
                    nc.tensor.matmul(
                        w[0:64, 0:64],
                        lhsT=ones_sb[:, 0:64],
                        rhs=ones_sb[:, 0:64],
                        start=True,
                        stop=True,
                        skip_group_check=True,
                    )

            warm_burst(60)
            # pre-phase: k oc0 (heads 0,1), q oc0 only; the rest sprinkled
            k_proj(0, 0, split_evac=True)
            q_proj(0, 0)

            proj_jobs = [
                lambda: k_proj(0, 1),
                lambda: v_proj(0),
                lambda: k_proj(1, 0),
                lambda: k_proj(1, 1),
                lambda: v_proj(1),
                lambda: q_proj(1, 0),
                lambda: v_proj(2),
                lambda: v_proj(3),
                lambda: q_proj(0, 1),
                lambda: v_proj(4),
                lambda: v_proj(5),
                lambda: q_proj(1, 1),
                lambda: v_proj(6),
                lambda: v_proj(7),
            ]

            def xa_tile():
                return psum.tile([128, 512], f32, tag="xa", bufs=2, name="xa")

            for b in range(8):
                if b == 2:
                    load_mask(1)
                alloc_block_bufs(b)
                for p in range(8):
                    emit_scores_pair(b, p)
                    if b > 0:
                        if p == 0:
                            xa_of[b - 1] = xa_tile()
                            if b >= 2:
                                emit_div_bcast(b - 2)
                        if p == 1:
                            if b >= 2:
                                emit_div_mult(b - 2)
                        emit_pv_pair(b - 1, p)
                        if p == 7:
                            r_of[b - 1] = emit_recip(b - 1)
                    if b == 7:
                        if p >= 3:
                            if p == 3:
                                xa_of[7] = xa_tile()
                            emit_pv_pair(7, p - 3)
                    if proj_jobs:
                        proj_jobs.pop(0)()
                # out-proj (oc, g0) needs xmult of blocks 0..3: emitted b5 p1
                if b == 5:
                    out_proj(0, 0)
                if b == 6:
                    out_proj(1, 0)
            # drain
            emit_div_bcast(6)
            emit_div_mult(6)
            for p in range(5, 8):
                emit_pv_pair(7, p)
            # partial out-proj for g1: everything except the h3 rows of x_sb[1]
            # (h3 = block 7 divides last); its 64-row matmul lands after.
            g0 = 512
            ps_oc = {}
            for oc in range(2):
                ps = st_tile()
                ps_oc[oc] = ps
                # bias folded in as a ones-row so the tail evacs are plain
                # copies that can run on Act and DVE in parallel
                nc.tensor.matmul(
                    ps[:, 0:512],
                    lhsT=bmr_sb[:, oc * 128 : (oc + 1) * 128],
                    rhs=ones_sb,
                    start=True,
                    stop=False,
                )
                nc.tensor.matmul(
                    ps[:, 0:512],
                    lhsT=wm_sb[0][:, oc * 128 : (oc + 1) * 128],
                    rhs=x_sb[0][:, g0 : g0 + 512],
                    start=False,
                    stop=False,
                )
                nc.tensor.matmul(
                    ps[:, 0:512],
                    lhsT=wm_sb[1][0:64, oc * 128 : (oc + 1) * 128],
                    rhs=x_sb[1][0:64, g0 : g0 + 512],
                    start=False,
                    stop=False,
                )
            r_of[7] = emit_recip(7)
            emit_div_bcast(7)
            emit_div_mult(7)
            for oc in range(2):
                nc.tensor.matmul(
                    ps_oc[oc][:, 0:512],
                    lhsT=wm_sb[1][64:128, oc * 128 : (oc + 1) * 128],
                    rhs=x_sb[1][64:128, g0 : g0 + 512],
                    start=False,
                    stop=True,
                )
            # stage both oc halves in one tile -> ONE fused final DMA
            # (each extra DMA instr costs ~625ns serial HWDGE on the tail)
            ot2 = work.tile([128, 1024], f16, tag="ot2", bufs=1, name="ot2")
            nc.scalar.activation(out=ot2[:, 0:512], in_=ps_oc[0][:, 0:512], func=Act.Copy)
            nc.vector.tensor_copy(out=ot2[:, 512:1024], in_=ps_oc[1][:, 0:512])
            dst = out_d[:, :].rearrange("(oc p) n -> p oc n", oc=2)[:, :, g0 : g0 + 512]
            nc.sync.dma_start(
                out=dst, in_=ot2[:, :].rearrange("p (oc n) -> p oc n", oc=2)
            )
    return nc


def _legalize_multi_waits(j):
    """Split >1 sync-waits per instruction into standalone EventSemaphore ops
    (this walrus build accepts at most one wait per TPB instruction)."""
    ctr = 0
    for f in j["functions"]:
        for b in f["blocks"]:
            out = []
            for inst in b["instructions"]:
                si = inst.get("sync_info")
                ow = (si or {}).get("on_wait") or []
                if len(ow) > 1:
                    for w in ow[:-1]:
                        ctr += 1
                        out.append(
                            {
                                "debug": inst.get("debug", 0),
                                "engine": inst["engine"],
                                "ins": [],
                                "name": f"legwait-{ctr}",
                                "opcode": "EventSemaphore",
                                "outs": [],
                                "sync_info": {"on_update": [], "on_wait": [w]},
                            }
                        )
                    si["on_wait"] = [ow[-1]]
                out.append(inst)
            b["instructions"] = out
    return j


def _get_nc():
    global _NC
    if _NC is None:
        import json as _json
        import types as _types

        nc = _build_nc()
        raw = nc.to_json_bytes()
        fixed = _json.dumps(_legalize_multi_waits(_json.loads(raw))).encode()
        nc.to_json_bytes = _types.MethodType(lambda self: fixed, nc)
        _NC = nc
    return _NC


def _prep_shards(inputs):
    f = lambda k: np.asarray(inputs[k], dtype=np.float32)
    q, k, v = f("query"), f("key"), f("value")
    w, mask = f("weight"), f("mask")
    Wq, bq = f("Wq"), f("bq")
    Wk, bk = f("Wk"), f("bk")
    Wv, bv = f("Wv"), f("bv")
    Wm, bm = f("Wm"), f("bm")

    def fuse(x):
        # [256, W] -> [128, 2*W] with [p, ic*W + w] = x[ic*128 + p, w]
        W = x.shape[1]
        return np.ascontiguousarray(
            x.reshape(2, 128, W).transpose(1, 0, 2).reshape(128, 2 * W)
        )

    p = _PERM
    wq2 = fuse((Wq[p] / 8.0).T.astype(F16))
    wk2 = fuse(Wk[p].T.astype(F16))
    WvTp = Wv[p].T
    wvT = np.zeros((D, VA), np.float32)
    for h in range(H):
        wvT[:, AUG * h : AUG * h + HD] = WvTp[:, HD * h : HD * (h + 1)]
    wv2 = fuse(wvT.astype(F16))
    wm2 = fuse(Wm[:, p].T.astype(F16))
    bqT = (bq[p] / 8.0).reshape(2, 128).T
    bkT = bk[p].reshape(2, 128).T
    # sum_m prob = 1, so the v bias is additive on x: fold into bm
    bmp = bm + Wm @ bv  # sum_m prob = 1 -> v bias is additive on x
    bmT = bmp.reshape(2, 128).T
    bias6 = np.ascontiguousarray(np.concatenate([bqT, bkT, bmT], axis=1))
    bmrow = np.ascontiguousarray(bmp.reshape(1, D)).astype(F16)

    qh, kh, vh = q.astype(F16), k.astype(F16), v.astype(F16)

    # per-batch tensors are shared by the two cores of each batch
    xk2_of = [fuse(kh[b]) for b in range(B)]
    xv2_of = [fuse(vh[b]) for b in range(B)]
    mwb_of = [(mask[b].T * w[b][:, None]).astype(F16) for b in range(B)]

    in_maps = []
    for c in range(NCORES):
        b, half = c // 2, c % 2
        n0 = half * NH
        sl = mwb_of[b][:, n0 : n0 + NH].reshape(16, 128, 2, 512)
        mw = np.ascontiguousarray(sl.transpose(1, 2, 0, 3)).reshape(128, 2 * 16 * 512)
        in_maps.append(
            dict(
                xq2=fuse(np.ascontiguousarray(qh[b, :, n0 : n0 + NH])),
                xk2=xk2_of[b],
                xv2=xv2_of[b],
                mw=mw,
                wq2=wq2,
                wk2=wk2,
                wv2=wv2,
                wm2=wm2,
                bias6=bias6,
                bmrow=bmrow,
            )
        )
    return in_maps


LAST_RESULT = None


def kernel(**inputs) -> np.ndarray:
    from concourse.bass_utils import run_bass_kernel_spmd

    in_maps = _prep_shards(inputs)
    nc = _get_nc()
    global LAST_RESULT
    LAST_RESULT = run_bass_kernel_spmd(nc, in_maps, core_ids=list(range(NCORES)))
    out = np.empty((B, D, N), np.float32)
    for c in range(NCORES):
        b, half = c // 2, c % 2
        out[b, :, half * NH : (half + 1) * NH] = LAST_RESULT.results[c]["out"].astype(
            np.float32
        )
    return out
